# revision 1
# baseline (speedup 1.0000x reference)
"""Chamfer distance loss kernel v3 for Trainium2 (8 NeuronCores, SPMD).

Math: for each batch m, M[i,j] = |t_i|^2 + |s_j|^2 - 2 t_i.s_j  (squared dists)
  dist1 = mean_j sqrt(min_i M), dist2 = mean_i sqrt(min_j M), out = (d1+d2)/2.

v3 design (vs v1's dual-emission + ACT-drain-everything, which was
Activation-engine bound at ~480us):
  - Emit the 32 i-tiles [128 i, 4096 j] ONCE (split-fp16 K=15 matmul), in two
    [128, 2048] fp32 PSUM halves. Half the PE work of v1.
  - ACT drains each half to an SBUF bf16 stage (the only PSUM exit).
  - DVE (sole tensor-tensor-min engine on TRN2; fused reduce ops
    TensorTensorReduce/TensorMaskReduce fault this runtime, and GPSIMD
    rejects TensorTensor at the ISA level):
      * dist1 col-acc: accD = min(accD, stage)   (tt, 2x bf16 mode)
      * dist2 fold chain: stage -> 2048 -> 1024 -> 512 (tt 2x) into a
        per-8-tile group buffer; one [128, 8, 512] tensor_reduce per group.
  - dist1 final: PE-transpose accD 128x128 blocks -> PSUM bf16, DVE group
    min-reduce; deferred into the next batch's loop to overlap the tail.
"""

import numpy as np

M_BATCH = 16
N = 4096
D = 3
N_CORES = 8
NB = M_BATCH // N_CORES  # batches per core
P = 128
IT = N // P  # 32 i-tiles
K_AUG = 15

# in-kernel repetition count (measurement only; 1 for production)
LOOP_REPS = 1

_CACHE = {}


def _build_nc(loop_reps=None, stage_bufs=6):
    import concourse.bacc as bacc
    import concourse.tile as tile
    from concourse import mybir
    from concourse.masks import make_identity
    from contextlib import ExitStack, nullcontext

    if loop_reps is None:
        loop_reps = LOOP_REPS

    F32 = mybir.dt.float32
    BF16 = mybir.dt.bfloat16
    FP16 = mybir.dt.float16
    X = mybir.AxisListType.X
    MIN = mybir.AluOpType.min

    HW = 2048  # psum half width (fp32, 4 banks)
    GRP = 8    # tiles per dist2 group reduce

    nc = bacc.Bacc("TRN2", target_bir_lowering=False)
    lhsT_d = nc.declare_dram_parameter("lhsT", [NB, K_AUG, N], FP16, isOutput=False)
    rhs_d = nc.declare_dram_parameter("rhs", [NB, K_AUG, N], FP16, isOutput=False)
    # mins[b, 0]: colmins (dist1), j indexed as [j%128, j//128]
    # mins[b, 1]: rowmins (dist2), i indexed as [i%128, i//128]
    mins_d = nc.declare_dram_parameter("mins", [NB, 2, P, IT], F32, isOutput=True)

    with ExitStack() as ctx:
        tc = ctx.enter_context(tile.TileContext(nc))
        consts = ctx.enter_context(tc.tile_pool(name="consts", bufs=1))
        inputs = ctx.enter_context(tc.tile_pool(name="inputs", bufs=2))
        stages = ctx.enter_context(tc.tile_pool(name="stages", bufs=stage_bufs))
        accs = ctx.enter_context(tc.tile_pool(name="accs", bufs=2))
        scr = ctx.enter_context(tc.tile_pool(name="scr", bufs=4))
        rowf = ctx.enter_context(tc.tile_pool(name="rowf", bufs=2))
        outs = ctx.enter_context(tc.tile_pool(name="outs", bufs=2))
        psum = ctx.enter_context(tc.tile_pool(name="psum", bufs=2, space="PSUM"))

        ident = consts.tile([P, P], BF16)
        make_identity(nc, ident)

        def finals(fctx):
            """Batch-final dist1 partition reduce (deferred into next batch)."""
            accD, colmins, rowmins, b = fctx
            for c8 in range(IT // 8):
                tp = psum.tile([P, 8, P], BF16, tag="mm")
                for k in range(8):
                    nc.tensor.transpose(
                        tp[:, k, :],
                        accD[:, (c8 * 8 + k) * P : (c8 * 8 + k + 1) * P],
                        ident,
                    )
                nc.vector.tensor_reduce(
                    out=colmins[:, c8 * 8 : (c8 + 1) * 8], in_=tp, axis=X, op=MIN
                )
            nc.sync.dma_start(out=mins_d[b, 0], in_=colmins)
            nc.sync.dma_start(out=mins_d[b, 1], in_=rowmins)

        loop_ctx = tc.For_i(0, loop_reps, 1) if loop_reps > 1 else nullcontext()
        with loop_ctx:
          pending = None
          for b in range(NB):
            lhsT_s = inputs.tile([K_AUG, N], FP16, tag="lhsT")
            rhs_s = inputs.tile([K_AUG, N], FP16, tag="rhs")
            nc.sync.dma_start(out=lhsT_s, in_=lhsT_d[b])
            nc.sync.dma_start(out=rhs_s, in_=rhs_d[b])

            accD = accs.tile([P, N], BF16, tag="accD")
            rowmins = outs.tile([P, IT], F32, tag="rowmins")
            colmins = outs.tile([P, IT], F32, tag="colmins")

            first_d = [None]
            rowfold = None

            for t in range(IT):
                if t == 3 and pending is not None:
                    finals(pending)
                    pending = None
                if t % GRP == 0:
                    rowfold = rowf.tile([P, GRP, 512], BF16, tag="rowfold")
                stage = stages.tile([P, N], BF16, tag="stage")
                for h in range(2):
                    ps = psum.tile([P, HW], F32, tag="mm")
                    for q in range(HW // 512):
                        nc.tensor.matmul(
                            ps[:, q * 512 : (q + 1) * 512],
                            lhsT_s[:, t * P : (t + 1) * P],
                            rhs_s[:, h * HW + q * 512 : h * HW + (q + 1) * 512],
                            start=True,
                            stop=True,
                        )
                    nc.scalar.copy(out=stage[:, h * HW : (h + 1) * HW], in_=ps)
                # dist1 col-acc chain (fuse the first pair into one op)
                if first_d[0] is None:
                    first_d[0] = stage
                elif first_d[0] is not False:
                    nc.vector.tensor_tensor(accD, first_d[0], stage, MIN)
                    first_d[0] = False
                else:
                    nc.vector.tensor_tensor(accD, stage, accD, MIN)
                # dist2 fold chain 4096 -> 512 at 2x
                f1 = scr.tile([P, N // 2], BF16, tag="f1")
                nc.vector.tensor_tensor(
                    f1, stage[:, 0 : N // 2], stage[:, N // 2 : N], MIN
                )
                f2 = scr.tile([P, N // 4], BF16, tag="f2")
                nc.vector.tensor_tensor(
                    f2, f1[:, 0 : N // 4], f1[:, N // 4 : N // 2], MIN
                )
                nc.vector.tensor_tensor(
                    rowfold[:, t % GRP, :],
                    f2[:, 0 : N // 8],
                    f2[:, N // 8 : N // 4],
                    MIN,
                )
                if t % GRP == GRP - 1:
                    nc.vector.tensor_reduce(
                        out=rowmins[:, t - GRP + 1 : t + 1],
                        in_=rowfold,
                        axis=X,
                        op=MIN,
                    )

            pending = (accD, colmins, rowmins, b)
          if pending is not None:
            finals(pending)

    nc.compile()
    return nc


def _get_nc():
    key = ("nc", LOOP_REPS)
    if key not in _CACHE:
        _CACHE[key] = _build_nc()
    return _CACHE[key]


def _prep_inputs(template, source):
    """Build split-fp16 augmented [m, 15, n] operands (same as v1)."""
    t = np.ascontiguousarray(template, dtype=np.float32)
    s = np.ascontiguousarray(source, dtype=np.float32)

    f16 = np.float16

    def split2(x):
        h = x.astype(f16).astype(np.float32)
        l = (x - h).astype(f16).astype(np.float32)
        return h, l

    def split3(x):
        h = x.astype(f16).astype(np.float32)
        r = x - h
        m = r.astype(f16).astype(np.float32)
        l = (r - m).astype(f16).astype(np.float32)
        return h, m, l

    ah, al = split2(t)  # [m, n, 3]
    bh, bl = split2(s)
    a2 = (t.astype(np.float64) ** 2).sum(-1).astype(np.float32)  # [m, n]
    b2 = (s.astype(np.float64) ** 2).sum(-1).astype(np.float32)
    a2h, a2m, a2l = split3(a2)
    b2h, b2m, b2l = split3(b2)
    ones = np.ones_like(a2)

    lrows = []
    rrows = []
    for c in range(3):
        lrows += [-2.0 * ah[..., c], (-2.0 / 32.0) * ah[..., c], -128.0 * al[..., c]]
        rrows += [bh[..., c], 32.0 * bl[..., c], bh[..., c] / 64.0]
    lrows += [a2h, 32.0 * a2m, 2048.0 * a2l, ones, ones / 32.0, ones / 2048.0]
    rrows += [ones, ones / 32.0, ones / 2048.0, b2h, 32.0 * b2m, 2048.0 * b2l]

    lhsT = np.stack(lrows, axis=1).astype(f16)  # [m, 15, n]
    rhs = np.stack(rrows, axis=1).astype(f16)
    return np.ascontiguousarray(lhsT), np.ascontiguousarray(rhs)


def run(template, source, trace=False):
    """Returns (result_scalar, exec_time_ns_or_None)."""
    from concourse import bass_utils

    nc = _get_nc()
    lhsT, rhs = _prep_inputs(template, source)
    in_maps = [
        {
            "lhsT": np.ascontiguousarray(lhsT[c * NB : (c + 1) * NB]),
            "rhs": np.ascontiguousarray(rhs[c * NB : (c + 1) * NB]),
        }
        for c in range(N_CORES)
    ]
    res = bass_utils.run_bass_kernel_spmd(
        nc, in_maps, core_ids=list(range(N_CORES)), trace=trace
    )
    mins = np.stack([r["mins"] for r in res.results])  # [8, NB, 2, P, IT]
    total = np.sqrt(np.maximum(mins.astype(np.float64), 0.0)).sum()
    out = np.float32(total / (2.0 * M_BATCH * N))
    return out, res.exec_time_ns


def kernel(template, source):
    out, _ = run(template, source, trace=False)
    return out

